# revision 11
# baseline (speedup 1.0000x reference)
"""LoRA multi-head attention on 8 TRN2 NeuronCores.

Sharding: data-parallel over batch (B=8 -> 1 batch element per core),
weights replicated, no collectives. Per-core kernel computes the full
attention block for one (1024, 1024) slice in bf16 with fp32 PSUM
accumulation, in "transposed activation space":

  xT, WqT/WkT/WvT/WoT built via bf16 DRAM-scratch (gpsimd cast DMA)
  + HWDGE DMA-transpose loads.
  qT = (Wq x^T + Bq Aq x^T / 16) / 8      [dout, n]  (1/8 = head scale)
  kT = Wk x^T + Bk Ak x^T / 16            [dout, n]
  v  = x Wv^T + ... (natural [n, dout]), stored per-head with a ones
       column appended ([v_h | 1]) so the PV matmul also produces the
       softmax denominators.
  per head: S^T[m,n] = kT_h^T qT_h ; P^T = exp(S^T) (no max-sub needed,
       |s| is O(4)); O^T/den via [v|1] matmul; normalize with
       reciprocal + ones-outer-product broadcast; assemble attnT.
  out = attnT^T WoT + lora + bo (bias via K=1 ones matmul into PSUM).
"""

import sys

if "/opt/trn_rl_repo" not in sys.path:
    sys.path.insert(0, "/opt/trn_rl_repo")

import numpy as np

N = 1024  # tokens
D = 1024  # model dim
H = 16    # heads
HD = 64   # head dim
R = 16    # lora rank
P = 128   # partitions
F = 512   # psum free-dim tile
NCORES = 8
SCALING = 1.0 / 16.0  # lora alpha/rank
SCALE = HD ** -0.5

_CACHE = {}


def _build():
    import concourse.bacc as bacc
    import concourse.mybir as mybir
    import concourse.tile as tile

    f32 = mybir.dt.float32
    bf16 = mybir.dt.bfloat16
    Exp = mybir.ActivationFunctionType.Exp

    nc = bacc.Bacc("TRN2", target_bir_lowering=False, debug=True)

    x_e = nc.declare_dram_parameter("x", [N, D], f32, isOutput=False)
    w_e = {
        nm: nc.declare_dram_parameter(nm, [D, D], f32, isOutput=False)
        for nm in ("Wq", "Wk", "Wv", "Wo")
    }
    bo_e = nc.declare_dram_parameter("bo", [D], f32, isOutput=False)
    a_e = {
        nm: nc.declare_dram_parameter(nm, [R, D], f32, isOutput=False)
        for nm in ("Aq", "Ak", "Av", "Ao")
    }
    b_e = {
        nm: nc.declare_dram_parameter(nm, [D, R], f32, isOutput=False)
        for nm in ("Bq", "Bk", "Bv", "Bo")
    }
    out_e = nc.declare_dram_parameter("out", [N, D], f32, isOutput=True)

    with tile.TileContext(nc) as tc:
        with (
            tc.tile_pool(name="wpool", bufs=1) as wpool,
            tc.tile_pool(name="dram", bufs=1, space="DRAM") as dram,
            tc.tile_pool(name="stage", bufs=2) as stage,
            tc.tile_pool(name="ps", bufs=1, space="PSUM") as ps,
        ):
            outT = {}
            v_sb = []
            with tc.tile_pool(name="apool", bufs=1) as apool:
                # ---- prep: transposed bf16 copies of x, Wq, Wk, Wv, Wo ----
                big = {"x": x_e, "Wq": w_e["Wq"], "Wk": w_e["Wk"],
                       "Wv": w_e["Wv"], "Wo": w_e["Wo"]}
                T = {}
                for nm, ext in big.items():
                    pool = wpool if nm == "Wo" else apool
                    scr = dram.tile([D, D], bf16, tag=f"scr_{nm}")
                    # cast f32 -> bf16, DRAM -> DRAM (SWDGE casts)
                    nc.gpsimd.dma_start(out=scr[:], in_=ext[:, :])
                    tiles = []
                    for t in range(8):
                        tt = pool.tile([P, D], bf16, tag=f"T_{nm}_{t}",
                                       name=f"T_{nm}_{t}")
                        nc.sync.dma_start(out=tt[:],
                                          in_=scr[:, t * P:(t + 1) * P],
                                          transpose=True)
                        tiles.append(tt)
                    T[nm] = tiles

                # ---- prep: small lora weights ----
                # A^T via bf16 DRAM scratch + xbar transpose (16-row src ok)
                aT = {}  # A^T: 8 tiles [128, 16] bf16 per name
                for nm, ext in a_e.items():
                    pool = wpool if nm == "Ao" else apool
                    scr = dram.tile([R, D], bf16, tag=f"scr_{nm}")
                    nc.gpsimd.dma_start(out=scr[:], in_=ext[:, :])
                    tiles = []
                    for t in range(8):
                        tt = pool.tile([P, R], bf16, tag=f"aT_{nm}_{t}",
                                       name=f"aT_{nm}_{t}")
                        nc.sync.dma_start(out=tt[:],
                                          in_=scr[:, t * P:(t + 1) * P],
                                          transpose=True)
                        tiles.append(tt)
                    aT[nm] = tiles
                # identity for PE-transposes of B chunks
                from concourse.masks import make_identity
                ident = apool.tile([P, P], bf16, tag="ident")
                make_identity(nc, ident[:])
                bT = {}  # B^T: [16, 1024] bf16 via PE transpose of chunks
                for nm, ext in b_e.items():
                    pool = wpool if nm == "Bo" else apool
                    scr = dram.tile([D, R], bf16, tag=f"scr_{nm}")
                    nc.gpsimd.dma_start(out=scr[:], in_=ext[:, :])
                    tt = pool.tile([R, D], bf16, tag=f"bT_{nm}",
                                   name=f"bT_{nm}")
                    for t in range(8):
                        bstage = stage.tile([P, R], bf16, tag="bstage")
                        nc.sync.dma_start(out=bstage[:],
                                          in_=scr[t * P:(t + 1) * P, :])
                        pts = ps.tile([R, P], bf16, tag="tpsum", bufs=1)
                        nc.tensor.transpose(pts[:], bstage[:], ident[:])
                        nc.scalar.copy(tt[:, t * P:(t + 1) * P], pts[:])
                    bT[nm] = tt
                # bias + ones helpers
                bo_sb = wpool.tile([1, D], bf16, tag="bo")
                nc.gpsimd.dma_start(out=bo_sb[:], in_=bo_e[None, :])
                ones128 = wpool.tile([1, P], bf16, tag="ones128")
                nc.vector.memset(ones128[:], 1.0)
                onesf = wpool.tile([P, HD], f32, tag="onesf")
                nc.vector.memset(onesf[:], 1.0)

                # ---- phase A: qT, kT ----
                for nm, wnm, anm, bnm, scl in (
                    ("q", "Wq", "Aq", "Bq", SCALE),
                    ("k", "Wk", "Ak", "Bk", None),
                ):
                    dst = [wpool.tile([P, D], bf16, tag=f"{nm}T_{t}",
                                      name=f"{nm}T_{t}") for t in range(8)]
                    for nh in range(2):
                        ns = slice(nh * F, (nh + 1) * F)
                        pt = ps.tile([R, F], f32, tag="tpsum", bufs=1)
                        for kt in range(8):
                            nc.tensor.matmul(pt[:], aT[anm][kt][:],
                                             T["x"][kt][:, ns],
                                             start=(kt == 0), stop=(kt == 7))
                        tsb = stage.tile([R, F], bf16, tag="tsb")
                        nc.scalar.mul(tsb[:], pt[:], SCALING)
                        for dt in range(8):
                            pq = ps.tile([P, F], f32, tag="projpsum", bufs=2)
                            for kt in range(8):
                                nc.tensor.matmul(
                                    pq[:], T[wnm][kt][:, dt * P:(dt + 1) * P],
                                    T["x"][kt][:, ns],
                                    start=(kt == 0), stop=False)
                            nc.tensor.matmul(pq[:],
                                             bT[bnm][:, dt * P:(dt + 1) * P],
                                             tsb[:], start=False, stop=True)
                            if scl is None:
                                nc.scalar.copy(dst[dt][:, ns], pq[:])
                            else:
                                nc.scalar.mul(dst[dt][:, ns], pq[:], scl)
                    outT[nm] = dst

                # ---- phase A: v natural, per-head layout [v_h | 1] ----
                tv = apool.tile([R, D], bf16, tag="tvT")
                for nh in range(2):
                    ns = slice(nh * F, (nh + 1) * F)
                    pt = ps.tile([R, F], f32, tag="tpsum", bufs=1)
                    for kt in range(8):
                        nc.tensor.matmul(pt[:], aT["Av"][kt][:],
                                         T["x"][kt][:, ns],
                                         start=(kt == 0), stop=(kt == 7))
                    nc.scalar.mul(tv[:, ns], pt[:], SCALING)
                VW = H * (HD + 1)  # 1040
                v_sb = [wpool.tile([P, VW], bf16, tag=f"v_{t}",
                                   name=f"v_{t}") for t in range(8)]
                for nt in range(8):
                    vr = v_sb[nt][:].rearrange("p (h c) -> p h c", c=HD + 1)
                    for dh in range(2):
                        ds = slice(dh * F, (dh + 1) * F)
                        pv = ps.tile([P, F], f32, tag="projpsum", bufs=2)
                        for kt in range(8):
                            nc.tensor.matmul(
                                pv[:], T["x"][kt][:, nt * P:(nt + 1) * P],
                                T["Wv"][kt][:, ds],
                                start=(kt == 0), stop=False)
                        nc.tensor.matmul(pv[:], tv[:, nt * P:(nt + 1) * P],
                                         bT["Bv"][:, ds],
                                         start=False, stop=True)
                        pvr = pv[:].rearrange("p (h c) -> p h c", c=HD)
                        nc.scalar.copy(vr[:, dh * 8:(dh + 1) * 8, 0:HD],
                                       pvr[:])
                    nc.vector.memset(vr[:, :, HD:HD + 1], 1.0)

            # ---- phase B: attention per head ----
            attnT = [wpool.tile([P, D], bf16, tag=f"attnT_{t}",
                                name=f"attnT_{t}") for t in range(8)]
            for h in range(H):
                qt = outT["q"][h // 2]
                ktt = outT["k"][h // 2]
                ro = (h % 2) * HD
                for nh in range(2):
                    ns = slice(nh * F, (nh + 1) * F)
                    po = ps.tile([HD + 1, F], f32, tag="pvpsum", bufs=2)
                    for mt in range(8):
                        psS = ps.tile([P, F], f32, tag="spsum", bufs=2)
                        nc.tensor.matmul(
                            psS[:], ktt[ro:ro + HD, mt * P:(mt + 1) * P],
                            qt[ro:ro + HD, ns], start=True, stop=True)
                        pte = stage.tile([P, F], bf16, tag="pt", bufs=3)
                        nc.scalar.activation(pte[:], psS[:], Exp)
                        nc.tensor.matmul(
                            po[:],
                            v_sb[mt][:, h * (HD + 1):(h + 1) * (HD + 1)],
                            pte[:], start=(mt == 0), stop=(mt == 7))
                    rden = stage.tile([P, F], f32, tag="rden")
                    nc.vector.reciprocal(rden[HD:HD + 1, :], po[HD:HD + 1, :])
                    pb = ps.tile([HD, F], f32, tag="bcast", bufs=1)
                    nc.tensor.matmul(pb[:], onesf[HD:HD + 1, :],
                                     rden[HD:HD + 1, :],
                                     start=True, stop=True)
                    pbs = stage.tile([HD, F], f32, tag="pbs")
                    nc.scalar.copy(pbs[:], pb[:])
                    ast = stage.tile([HD, F], bf16, tag="ast")
                    nc.vector.tensor_mul(ast[:], po[0:HD, :], pbs[:])
                    nc.sync.dma_start(out=attnT[h // 2][ro:ro + HD, ns],
                                      in_=ast[:])

            # ---- phase C: output projection ----
            to = wpool.tile([R, D], bf16, tag="toT")
            for nh in range(2):
                ns = slice(nh * F, (nh + 1) * F)
                pt = ps.tile([R, F], f32, tag="tpsum", bufs=1)
                for kt in range(8):
                    nc.tensor.matmul(pt[:], aT["Ao"][kt][:],
                                     attnT[kt][:, ns],
                                     start=(kt == 0), stop=(kt == 7))
                nc.scalar.mul(to[:, ns], pt[:], SCALING)
            for nt in range(8):
                for dh in range(2):
                    ds = slice(dh * F, (dh + 1) * F)
                    pf = ps.tile([P, F], f32, tag="projpsum", bufs=2)
                    nc.tensor.matmul(pf[:], ones128[:], bo_sb[:, ds],
                                     start=True, stop=False)
                    for kt in range(8):
                        nc.tensor.matmul(pf[:],
                                         attnT[kt][:, nt * P:(nt + 1) * P],
                                         T["Wo"][kt][:, ds],
                                         start=False, stop=False)
                    nc.tensor.matmul(pf[:], to[:, nt * P:(nt + 1) * P],
                                     bT["Bo"][:, ds], start=False, stop=True)
                    osb = stage.tile([P, F], f32, tag="osb")
                    nc.scalar.copy(osb[:], pf[:])
                    nc.sync.dma_start(out=out_e[nt * P:(nt + 1) * P, ds],
                                      in_=osb[:])
    nc.compile()
    return nc


def _get_nc():
    if "nc" not in _CACHE:
        _CACHE["nc"] = _build()
    return _CACHE["nc"]


def kernel(**inputs):
    from concourse import bass_utils

    nc = _get_nc()
    names = ["Wq", "Wk", "Wv", "Wo", "bo", "Aq", "Bq", "Ak", "Bk",
             "Av", "Bv", "Ao", "Bo"]
    shared = {nm: np.ascontiguousarray(np.asarray(inputs[nm], np.float32))
              for nm in names}
    x = np.ascontiguousarray(np.asarray(inputs["x"], np.float32))
    in_maps = [dict(shared, x=x[i]) for i in range(NCORES)]
    res = bass_utils.run_bass_kernel_spmd(nc, in_maps,
                                          core_ids=list(range(NCORES)))
    return np.stack([res.results[i]["out"] for i in range(NCORES)], axis=0)


# revision 13
# speedup vs baseline: 1.3737x; 1.3737x over previous
"""LoRA multi-head attention on 8 TRN2 NeuronCores.

Sharding: data-parallel over batch (B=8 -> 1 batch element per core),
weights replicated, no collectives. Per-core kernel computes the full
attention block for one (1024, 1024) slice in bf16 with fp32 PSUM
accumulation, in "transposed activation space":

  xT, WqT/WkT/WvT/WoT built via bf16 DRAM-scratch (gpsimd cast DMA)
  + HWDGE DMA-transpose loads.
  qT = (Wq x^T + Bq Aq x^T / 16) / 8      [dout, n]  (1/8 = head scale)
  kT = Wk x^T + Bk Ak x^T / 16            [dout, n]
  v  = x Wv^T + ... (natural [n, dout]), stored per-head with a ones
       column appended ([v_h | 1]) so the PV matmul also produces the
       softmax denominators.
  per head: S^T[m,n] = kT_h^T qT_h ; P^T = exp(S^T) (no max-sub needed,
       |s| is O(4)); O^T/den via [v|1] matmul; normalize with
       reciprocal + ones-outer-product broadcast; assemble attnT.
  out = attnT^T WoT + lora + bo (bias via K=1 ones matmul into PSUM).
"""

import sys

if "/opt/trn_rl_repo" not in sys.path:
    sys.path.insert(0, "/opt/trn_rl_repo")

import numpy as np

N = 1024  # tokens
D = 1024  # model dim
H = 16    # heads
HD = 64   # head dim
R = 16    # lora rank
P = 128   # partitions
F = 512   # psum free-dim tile
NCORES = 8
SCALING = 1.0 / 16.0  # lora alpha/rank
SCALE = HD ** -0.5

_CACHE = {}


def _build():
    import concourse.bacc as bacc
    import concourse.mybir as mybir
    import concourse.tile as tile

    f32 = mybir.dt.float32
    bf16 = mybir.dt.bfloat16
    Exp = mybir.ActivationFunctionType.Exp

    nc = bacc.Bacc("TRN2", target_bir_lowering=False, debug=True)

    x_e = nc.declare_dram_parameter("x", [N, D], f32, isOutput=False)
    w_e = {
        nm: nc.declare_dram_parameter(nm, [D, D], f32, isOutput=False)
        for nm in ("Wq", "Wk", "Wv", "Wo")
    }
    bo_e = nc.declare_dram_parameter("bo", [D], f32, isOutput=False)
    a_e = {
        nm: nc.declare_dram_parameter(nm, [R, D], f32, isOutput=False)
        for nm in ("Aq", "Ak", "Av", "Ao")
    }
    b_e = {
        nm: nc.declare_dram_parameter(nm, [D, R], f32, isOutput=False)
        for nm in ("Bq", "Bk", "Bv", "Bo")
    }
    out_e = nc.declare_dram_parameter("out", [N, D], f32, isOutput=True)

    with tile.TileContext(nc) as tc:
        with (
            tc.tile_pool(name="wpool", bufs=1) as wpool,
            tc.tile_pool(name="dram", bufs=1, space="DRAM") as dram,
            tc.tile_pool(name="stage", bufs=2) as stage,
            tc.tile_pool(name="ps", bufs=1, space="PSUM") as ps,
        ):
            outT = {}
            v_sb = []
            with tc.tile_pool(name="apool", bufs=1) as apool:
                # ---- prep: transposed bf16 copies of x, Wq, Wk, Wv, Wo ----
                big = {"x": x_e, "Wq": w_e["Wq"], "Wk": w_e["Wk"],
                       "Wv": w_e["Wv"], "Wo": w_e["Wo"]}
                T = {}
                for nm, ext in big.items():
                    pool = wpool if nm == "Wo" else apool
                    scr = dram.tile([D, D], bf16, tag=f"scr_{nm}")
                    # cast f32 -> bf16, DRAM -> DRAM (SWDGE casts)
                    nc.gpsimd.dma_start(out=scr[:], in_=ext[:, :])
                    tiles = []
                    for t in range(8):
                        tt = pool.tile([P, D], bf16, tag=f"T_{nm}_{t}",
                                       name=f"T_{nm}_{t}")
                        nc.sync.dma_start(out=tt[:],
                                          in_=scr[:, t * P:(t + 1) * P],
                                          transpose=True)
                        tiles.append(tt)
                    T[nm] = tiles

                # ---- prep: small lora weights ----
                # A^T via bf16 DRAM scratch + xbar transpose (16-row src ok)
                aT = {}  # A^T: 8 tiles [128, 16] bf16 per name
                for nm, ext in a_e.items():
                    pool = wpool if nm == "Ao" else apool
                    scr = dram.tile([R, D], bf16, tag=f"scr_{nm}")
                    nc.gpsimd.dma_start(out=scr[:], in_=ext[:, :])
                    tiles = []
                    for t in range(8):
                        tt = pool.tile([P, R], bf16, tag=f"aT_{nm}_{t}",
                                       name=f"aT_{nm}_{t}")
                        nc.sync.dma_start(out=tt[:],
                                          in_=scr[:, t * P:(t + 1) * P],
                                          transpose=True)
                        tiles.append(tt)
                    aT[nm] = tiles
                # identity for PE-transposes of B chunks
                from concourse.masks import make_identity
                ident = apool.tile([P, P], bf16, tag="ident")
                make_identity(nc, ident[:])
                bT = {}  # B^T: [16, 1024] bf16 via PE transpose of chunks
                for nm, ext in b_e.items():
                    pool = wpool if nm == "Bo" else apool
                    scr = dram.tile([D, R], bf16, tag=f"scr_{nm}")
                    nc.gpsimd.dma_start(out=scr[:], in_=ext[:, :])
                    tt = pool.tile([R, D], bf16, tag=f"bT_{nm}",
                                   name=f"bT_{nm}")
                    for t in range(8):
                        bstage = stage.tile([P, R], bf16, tag="bstage")
                        nc.sync.dma_start(out=bstage[:],
                                          in_=scr[t * P:(t + 1) * P, :])
                        pts = ps.tile([R, P], bf16, tag="tpsum", bufs=1)
                        nc.tensor.transpose(pts[:], bstage[:], ident[:])
                        nc.scalar.copy(tt[:, t * P:(t + 1) * P], pts[:])
                    bT[nm] = tt
                # bias + ones helpers
                bo_sb = wpool.tile([1, D], bf16, tag="bo")
                nc.gpsimd.dma_start(out=bo_sb[:], in_=bo_e[None, :])
                ones128 = wpool.tile([1, P], bf16, tag="ones128")
                nc.vector.memset(ones128[:], 1.0)
                onesf = wpool.tile([P, HD], f32, tag="onesf")
                nc.vector.memset(onesf[:], 1.0)

                # ---- phase A0: v natural first (so attention can start
                # as soon as each qT/kT tile pair lands) ----
                tv = apool.tile([R, D], bf16, tag="tvT")
                for nh in range(2):
                    ns = slice(nh * F, (nh + 1) * F)
                    pt = ps.tile([R, F], f32, tag="tpsum", bufs=1)
                    for kt in range(8):
                        nc.tensor.matmul(pt[:], aT["Av"][kt][:],
                                         T["x"][kt][:, ns],
                                         start=(kt == 0), stop=(kt == 7))
                    nc.vector.tensor_scalar_mul(tv[:, ns], pt[:], SCALING)
                VW = H * (HD + 1)  # 1040
                v_sb = [wpool.tile([P, VW], bf16, tag=f"v_{t}",
                                   name=f"v_{t}") for t in range(8)]
                for nt in range(8):
                    vr = v_sb[nt][:].rearrange("p (h c) -> p h c", c=HD + 1)
                    for dh in range(2):
                        ds = slice(dh * F, (dh + 1) * F)
                        pv = ps.tile([P, F], f32, tag="projpsum", bufs=2)
                        for kt in range(8):
                            nc.tensor.matmul(
                                pv[:], T["x"][kt][:, nt * P:(nt + 1) * P],
                                T["Wv"][kt][:, ds],
                                start=(kt == 0), stop=False)
                        nc.tensor.matmul(pv[:], tv[:, nt * P:(nt + 1) * P],
                                         bT["Bv"][:, ds],
                                         start=False, stop=True)
                        pvr = pv[:].rearrange("p (h c) -> p h c", c=HD)
                        nc.vector.tensor_copy(
                            vr[:, dh * 8:(dh + 1) * 8, 0:HD], pvr[:])
                    nc.vector.memset(vr[:, :, HD:HD + 1], 1.0)

                # ---- lora intermediates for q/k (live across dt loop) ----
                tsb = {}
                for nm, anm in (("q", "Aq"), ("k", "Ak")):
                    for nh in range(2):
                        ns = slice(nh * F, (nh + 1) * F)
                        pt = ps.tile([R, F], f32, tag="tpsum", bufs=1)
                        for kt in range(8):
                            nc.tensor.matmul(pt[:], aT[anm][kt][:],
                                             T["x"][kt][:, ns],
                                             start=(kt == 0), stop=(kt == 7))
                        t_s = stage.tile([R, F], bf16, tag="tsb", bufs=4,
                                         name=f"tsb_{nm}_{nh}")
                        nc.vector.tensor_scalar_mul(t_s[:], pt[:], SCALING)
                        tsb[(nm, nh)] = t_s

                # ---- phase A/B interleaved: per dout-tile dt, compute
                # qT[dt], kT[dt], then run attention for heads 2dt, 2dt+1 ----
                outT = {"q": [wpool.tile([P, D], bf16, tag=f"qT_{t}",
                                         name=f"qT_{t}") for t in range(8)],
                        "k": [wpool.tile([P, D], bf16, tag=f"kT_{t}",
                                         name=f"kT_{t}") for t in range(8)]}
                attnT = [wpool.tile([P, D], bf16, tag=f"attnT_{t}",
                                    name=f"attnT_{t}") for t in range(8)]
                for dt in range(8):
                    for nm, wnm, bnm, scl in (("q", "Wq", "Bq", SCALE),
                                              ("k", "Wk", "Bk", None)):
                        dst = outT[nm][dt]
                        for nh in range(2):
                            ns = slice(nh * F, (nh + 1) * F)
                            pq = ps.tile([P, F], f32, tag="projpsum", bufs=2)
                            for kt in range(8):
                                nc.tensor.matmul(
                                    pq[:], T[wnm][kt][:, dt * P:(dt + 1) * P],
                                    T["x"][kt][:, ns],
                                    start=(kt == 0), stop=False)
                            nc.tensor.matmul(pq[:],
                                             bT[bnm][:, dt * P:(dt + 1) * P],
                                             tsb[(nm, nh)][:],
                                             start=False, stop=True)
                            if scl is None:
                                nc.vector.tensor_copy(dst[:, ns], pq[:])
                            else:
                                nc.vector.tensor_scalar_mul(dst[:, ns],
                                                            pq[:], scl)
                    # attention for the two heads living in tile dt
                    for h in (2 * dt, 2 * dt + 1):
                        qt = outT["q"][dt]
                        ktt = outT["k"][dt]
                        ro = (h % 2) * HD
                        for nh in range(2):
                            ns = slice(nh * F, (nh + 1) * F)
                            po = ps.tile([HD + 1, F], f32, tag="pvpsum",
                                         bufs=2)
                            for mt in range(8):
                                psS = ps.tile([P, F], f32, tag="spsum",
                                              bufs=2)
                                nc.tensor.matmul(
                                    psS[:],
                                    ktt[ro:ro + HD, mt * P:(mt + 1) * P],
                                    qt[ro:ro + HD, ns],
                                    start=True, stop=True)
                                pte = stage.tile([P, F], bf16, tag="pt",
                                                 bufs=3)
                                nc.scalar.activation(pte[:], psS[:], Exp)
                                nc.tensor.matmul(
                                    po[:],
                                    v_sb[mt][:, h * (HD + 1):
                                             (h + 1) * (HD + 1)],
                                    pte[:], start=(mt == 0), stop=(mt == 7))
                            den = stage.tile([P, F], f32, tag="rden")
                            nc.vector.tensor_copy(den[HD:HD + 1, :],
                                                  po[HD:HD + 1, :])
                            pb = ps.tile([HD, F], f32, tag="bcast", bufs=1)
                            nc.tensor.matmul(pb[:], onesf[HD:HD + 1, :],
                                             den[HD:HD + 1, :],
                                             start=True, stop=True)
                            pbs = stage.tile([HD, F], f32, tag="pbs")
                            nc.vector.reciprocal(pbs[:], pb[:])
                            ast = stage.tile([HD, F], bf16, tag="ast")
                            nc.vector.tensor_mul(ast[:], po[0:HD, :], pbs[:])
                            nc.sync.dma_start(
                                out=attnT[dt][ro:ro + HD, ns], in_=ast[:])

            # ---- phase C: output projection ----
            to = wpool.tile([R, D], bf16, tag="toT")
            for nh in range(2):
                ns = slice(nh * F, (nh + 1) * F)
                pt = ps.tile([R, F], f32, tag="tpsum", bufs=1)
                for kt in range(8):
                    nc.tensor.matmul(pt[:], aT["Ao"][kt][:],
                                     attnT[kt][:, ns],
                                     start=(kt == 0), stop=(kt == 7))
                nc.scalar.mul(to[:, ns], pt[:], SCALING)
            for nt in range(8):
                for dh in range(2):
                    ds = slice(dh * F, (dh + 1) * F)
                    pf = ps.tile([P, F], f32, tag="projpsum", bufs=2)
                    nc.tensor.matmul(pf[:], ones128[:], bo_sb[:, ds],
                                     start=True, stop=False)
                    for kt in range(8):
                        nc.tensor.matmul(pf[:],
                                         attnT[kt][:, nt * P:(nt + 1) * P],
                                         T["Wo"][kt][:, ds],
                                         start=False, stop=False)
                    nc.tensor.matmul(pf[:], to[:, nt * P:(nt + 1) * P],
                                     bT["Bo"][:, ds], start=False, stop=True)
                    osb = stage.tile([P, F], f32, tag="osb")
                    nc.vector.tensor_copy(osb[:], pf[:])
                    nc.sync.dma_start(out=out_e[nt * P:(nt + 1) * P, ds],
                                      in_=osb[:])
    nc.compile()
    return nc


def _get_nc():
    if "nc" not in _CACHE:
        _CACHE["nc"] = _build()
    return _CACHE["nc"]


def kernel(**inputs):
    from concourse import bass_utils

    nc = _get_nc()
    names = ["Wq", "Wk", "Wv", "Wo", "bo", "Aq", "Bq", "Ak", "Bk",
             "Av", "Bv", "Ao", "Bo"]
    shared = {nm: np.ascontiguousarray(np.asarray(inputs[nm], np.float32))
              for nm in names}
    x = np.ascontiguousarray(np.asarray(inputs["x"], np.float32))
    in_maps = [dict(shared, x=x[i]) for i in range(NCORES)]
    res = bass_utils.run_bass_kernel_spmd(nc, in_maps,
                                          core_ids=list(range(NCORES)))
    return np.stack([res.results[i]["out"] for i in range(NCORES)], axis=0)
